# revision 28
# baseline (speedup 1.0000x reference)
"""Multi-head causal attention on 8 Trainium2 NeuronCores.

Problem: X [2, 2048, 1024] f32, W_q/W_k/W_v [1024, 1024], W_o [1024, 1024],
b_o [1024]; 16 heads, head_dim 64, causal softmax attention + out projection.

Sharding: 2 (batch) x 4 (head-blocks of 4 heads) = 8 cores. Each core
computes q/k/v for its 4 heads on its batch, causal attention, and a partial
output projection ctx @ W_o[rows]. Host sums the 4 partials per batch and
adds b_o. No cross-core collectives.

Single-pass structure (one pool scope; the Tile scheduler interleaves, with
emission order as priority):
  - X arrives host-transposed in (qc, cb)-block-contiguous bf16 so XT tiles
    are single contiguous DMA loads; weights are one wide DMA each, all
    ordered by first need (front-critical on sync, bulk deferred / scalar).
    W_q carries the 1/sqrt(d) softmax scale.
  - QKV projection and output-projection work is queued as fine-grained
    (<=2 matmul) closures popped one per attention k-block iteration, so
    the in-order PE queue interleaves them into the scores->exp->AV stream
    (exp on ScalarE, (N+352)/1.2 ns, is the attention-stream floor);
    drain_until gates guarantee producers are emitted before consumers.
  - Scores: row-packed concurrent K=64 matmul pairs (tile_position 0/64)
    into [128,1024] PSUM; strip mask added on DVE for diagonal blocks; one
    full-width exp per k-block.
  - AV: vt [vA|ones|pad|ones|pad|vB] sliding-window trick accumulates both
    heads' ctx and the softmax denominators in two PSUM banks, skipping
    fully-masked columns.
  - Normalization deferred off the critical path (one DVE-op per iteration):
    den rows -> chunked DVE reciprocal (ACT ln/exp(-x) for the final chunk,
    where ACT is idle) -> DRAM round-trip broadcast -> in-place bf16 muls
    on the unnormalized ctxn; outproj tiles become PE filler when a
    q-chunk's two pairs are normalized.
  - PSUM: s x2 (4 banks) + ctx1 + ctx2 + fill x2 (qkv/outproj shared,
    double-buffered) = 8 banks. ScalarE kept exp-only during the stream;
    copies pinned to DVE; vt ones/pads via gpsimd memset; output stores
    alternate between the sync and scalar DMA queues.
"""
import sys

sys.path.insert(0, "/opt/trn_rl_repo")

import numpy as np

NEG = -1.0e9
B, NTOK, DIN = 2, 2048, 1024
NH, HD = 16, 64
HPC = 4            # heads per core
CLOC = HPC * HD    # 256 local channels
NCORES = 8
NTB = NTOK // 128  # 16 token blocks
NQC = NTOK // 512  # 4 q-chunks
NCB = DIN // 128   # 8 contraction blocks

_CACHE = {}
_last_in_maps = None


def _build():
    from concourse import bacc, mybir, tile

    F32 = mybir.dt.float32
    BF16 = mybir.dt.bfloat16
    EXP = mybir.ActivationFunctionType.Exp
    P = 128

    nc = bacc.Bacc(None)
    # host-transposed X in (qc, cb)-block-contiguous layout: block (qc, cb)
    # at rows [(qc*NCB+cb)*128, +128) is XT[cb*128:(cb+1)*128, qc*512:+512]
    Xr = nc.declare_dram_parameter("Xr", [NQC * NCB * P, 512], BF16,
                                   isOutput=False)
    # weights cb-major along columns: [:, cb*256:(cb+1)*256] = W[cb block]
    Wq = nc.declare_dram_parameter("Wq", [P, NCB * CLOC], BF16, isOutput=False)
    Wk = nc.declare_dram_parameter("Wk", [P, NCB * CLOC], BF16, isOutput=False)
    Wv = nc.declare_dram_parameter("Wv", [P, NCB * CLOC], BF16, isOutput=False)
    Wo = nc.declare_dram_parameter("Wo", [CLOC, DIN], BF16, isOutput=False)
    tri = nc.declare_dram_parameter("tri", [P, P], F32, isOutput=False)
    out = nc.declare_dram_parameter("out", [NTOK, DIN], BF16, isOutput=True)

    with tile.TileContext(nc) as tc:
        with (
            tc.tile_pool(name="const", bufs=1) as constp,
            tc.tile_pool(name="xt", bufs=1) as xtp,
            tc.tile_pool(name="w", bufs=1) as wp,
            tc.tile_pool(name="qkT", bufs=1) as qkTp,
            tc.tile_pool(name="vt", bufs=1) as vtp,
            tc.tile_pool(name="ctxn", bufs=1) as ctxnp,
            tc.tile_pool(name="att", bufs=1) as attp,
            tc.tile_pool(name="osb", bufs=1) as osbp,
            tc.tile_pool(name="dsc", bufs=1, space="DRAM") as dscp,
            tc.tile_pool(name="ps", bufs=1, space="PSUM") as psp,
        ):
            tri_sb = constp.tile([P, P], F32, tag="tri")
            nc.sync.dma_start(tri_sb[:], tri[:])

            # weights: one wide tile per matrix, single contiguous DMA each
            wq_sb = wp.tile([P, NCB * CLOC], BF16, tag="wq", name="wq")
            wk_sb = wp.tile([P, NCB * CLOC], BF16, tag="wk", name="wk")
            wv_sb = wp.tile([P, NCB * CLOC], BF16, tag="wv", name="wv")
            wo_sb = [wp.tile([P, DIN], BF16, tag=f"wo{p}", name=f"wo{p}")
                     for p in range(2)]
            XT = [
                [xtp.tile([P, 512], BF16, tag=f"xt{cb}_{q}", name=f"xt{cb}_{q}")
                 for q in range(NQC)]
                for cb in range(NCB)
            ]

            def load_xt(q, eng):
                for cb in range(NCB):
                    r0 = (q * NCB + cb) * P
                    eng.dma_start(XT[cb][q][:], Xr[r0:r0 + P, :])

            # ordered by first need; front-critical loads on sync, bulk on
            # the scalar queue (idle until the first exp ~16us in)
            nc.sync.dma_start(wq_sb[:], Wq[:])
            nc.scalar.dma_start(wv_sb[:], Wv[:])
            load_xt(0, nc.sync)
            nc.sync.dma_start(wk_sb[:], Wk[:])
            for p in range(2):
                nc.scalar.dma_start(wo_sb[p][:], Wo[p * P:(p + 1) * P, :])
            load_xt(1, nc.sync)

            # static result tiles
            qT = [qkTp.tile([P, NTOK], BF16, tag=f"qT{p}", name=f"qT{p}")
                  for p in range(2)]
            kT = [qkTp.tile([P, NTOK], BF16, tag=f"kT{p}", name=f"kT{p}")
                  for p in range(2)]
            # v tiles [128 keys, 192]: [vA | ones | pad | ones | pad | vB]
            vt = [
                [vtp.tile([P, 192], BF16, tag=f"vt{p}_{tb}", name=f"vt{p}_{tb}")
                 for tb in range(NTB)]
                for p in range(2)
            ]
            ctxn = [
                [ctxnp.tile([P, 512], BF16, tag=f"ctxn{p}_{qc}",
                            name=f"ctxn{p}_{qc}") for qc in range(NQC)]
                for p in range(2)
            ]
            for p in range(2):
                for tb in range(NTB):
                    t = vt[p][tb]
                    nc.gpsimd.memset(t[:, 64:128], 0.0)
                    nc.gpsimd.memset(t[:, 64:65], 1.0)
                    nc.gpsimd.memset(t[:, 96:97], 1.0)

            # ---------------- emission helpers ----------------
            # pe_work / dve_work: queues of (key, closure); each closure is
            # <=2 matmuls (or one DVE op chain link) so the in-order engine
            # queues interleave finely with the scores/exp/AV stream.
            pe_work = []
            dve_work = []
            emitted = {}   # key -> remaining closures not yet emitted

            def push_pe(key, fn):
                pe_work.append((key, fn))
                emitted[key] = emitted.get(key, 0) + 1

            def pop_pe(n=1):
                for _ in range(n):
                    if not pe_work:
                        return
                    k, fn = pe_work.pop(0)
                    fn()
                    emitted[k] -= 1

            def pop_dve():
                if dve_work:
                    dve_work.pop(0)[1]()

            def drain_until(keys):
                need = [k for k in keys if emitted.get(k, 0) > 0]
                while need:
                    k, fn = pe_work.pop(0)
                    fn()
                    emitted[k] -= 1
                    need = [k for k in keys if emitted.get(k, 0) > 0]

            def emit_qk_pair(w_sb, dst, p, qc, quarter, state):
                if quarter == 0:
                    state["t"] = psp.tile([P, 512], F32, tag="fill", bufs=2,
                                          name="qk_ps")
                t = state["t"]
                for cb in range(2 * quarter, 2 * quarter + 2):
                    csl = slice(cb * CLOC + p * P, cb * CLOC + (p + 1) * P)
                    nc.tensor.matmul(
                        t[:], w_sb[:, csl], XT[cb][qc][:],
                        start=(cb == 0), stop=(cb == NCB - 1),
                    )
                if quarter == 3:
                    nc.vector.tensor_copy(
                        dst[p][:, qc * 512:(qc + 1) * 512], t[:]
                    )

            def emit_v_pair(tb, quarter, state):
                if quarter == 0:
                    state["t"] = psp.tile([P, 512], F32, tag="fill", bufs=2,
                                          name="v_ps")
                t = state["t"]
                for cb in range(2 * quarter, 2 * quarter + 2):
                    nc.tensor.matmul(
                        t[:, 0:CLOC],
                        XT[cb][tb // 4][:, (tb % 4) * P:(tb % 4 + 1) * P],
                        wv_sb[:, cb * CLOC:(cb + 1) * CLOC],
                        start=(cb == 0), stop=(cb == NCB - 1),
                    )
                if quarter == 3:
                    for p in range(2):
                        hA, hB = 2 * p, 2 * p + 1
                        nc.vector.tensor_copy(
                            vt[p][tb][:, 0:64], t[:, hA * 64:(hA + 1) * 64]
                        )
                        nc.vector.tensor_copy(
                            vt[p][tb][:, 128:192], t[:, hB * 64:(hB + 1) * 64]
                        )

            def push_qk(p, qc):
                for nm, w_sb, dst in (("q", wq_sb, qT), ("k", wk_sb, kT)):
                    st = {}
                    for quarter in range(4):
                        push_pe(
                            (nm, p, qc),
                            lambda w_sb=w_sb, dst=dst, p=p, qc=qc,
                            quarter=quarter, st=st:
                            emit_qk_pair(w_sb, dst, p, qc, quarter, st)
                        )

            def push_v(tb):
                st = {}
                for quarter in range(4):
                    push_pe(
                        ("v", tb),
                        lambda tb=tb, quarter=quarter, st=st:
                        emit_v_pair(tb, quarter, st)
                    )

            def emit_outproj_tile(qc, i, jc):
                tb = 4 * qc + i
                tsl = slice(tb * P, (tb + 1) * P)
                jsl = slice(jc * 512, (jc + 1) * 512)
                o_ps = psp.tile([P, 512], F32, tag="fill", bufs=2,
                                name="o_ps")
                for pp in range(2):
                    nc.tensor.matmul(
                        o_ps[:],
                        ctxn[pp][qc][:, i * P:(i + 1) * P],
                        wo_sb[pp][:, jsl],
                        start=(pp == 0), stop=(pp == 1),
                    )
                o_sb = osbp.tile([P, 512], BF16, tag="o_sb", bufs=4,
                                 name="o_sb")
                nc.vector.tensor_copy(o_sb[:], o_ps[:])
                (nc.sync if (i + jc) % 2 == 0 else nc.scalar).dma_start(
                    out[tsl, jsl], o_sb[:]
                )

            # norm bookkeeping: when both chunks of a qc have their norm
            # fully emitted, its outproj tiles become pe filler
            norm_done = [0, 0, 0, 0]

            def norm_complete(qc):
                norm_done[qc] += 1
                if norm_done[qc] == 2:
                    for i in range(4):
                        for jc in range(2):
                            push_pe(
                                ("o", qc, i, jc),
                                lambda qc=qc, i=i, jc=jc:
                                emit_outproj_tile(qc, i, jc)
                            )

            LOG = mybir.ActivationFunctionType.Ln

            def push_norm(den, p, qc):
                rec = attp.tile([P, 512], F32, tag="rec", bufs=2, name="rec")
                bc = attp.tile([P, 512], BF16, tag="bc", bufs=2, name="bc")
                d_t = dscp.tile([2, 512], BF16, tag="d", bufs=2, name="d_t")

                last = (p, qc) == (1, NQC - 1)
                recb = attp.tile([P, 512], BF16, tag="recb", bufs=2,
                                 name="recb")

                def recq(rec=rec, recb=recb, den=den, last=last):
                    if last:
                        # tail: ACT is idle; 1/d = exp(-ln d), 32-aligned
                        # windows (rows 33..63 / 65..95 are garbage, unread)
                        nc.scalar.activation(rec[32:64, :], den[32:64, :], LOG)
                        nc.scalar.activation(recb[32:64, :], rec[32:64, :],
                                             EXP, scale=-1.0)
                        nc.scalar.activation(rec[64:96, :], den[64:96, :], LOG)
                        nc.scalar.activation(recb[64:96, :], rec[64:96, :],
                                             EXP, scale=-1.0)
                    else:
                        for rc in range(4):
                            rsl = slice(rc * 128, (rc + 1) * 128)
                            nc.vector.reciprocal(rec[:, rsl], den[:, rsl])
                        nc.vector.tensor_copy(recb[64:65, :], rec[64:65, :])
                        nc.vector.tensor_copy(recb[32:33, :], rec[32:33, :])

                def dmas(recb=recb, d_t=d_t, bc=bc):
                    nc.sync.dma_start(d_t[0:1, :], recb[64:65, :])
                    nc.sync.dma_start(d_t[1:2, :], recb[32:33, :])
                    nc.sync.dma_start(
                        bc[0:64, :], d_t[0:1, :].to_broadcast((64, 512))
                    )
                    nc.sync.dma_start(
                        bc[64:128, :], d_t[1:2, :].to_broadcast((64, 512))
                    )

                def muls(bc=bc, p=p, qc=qc):
                    nc.vector.tensor_mul(
                        ctxn[p][qc][0:64, :], ctxn[p][qc][0:64, :],
                        bc[0:64, :],
                    )
                    nc.vector.tensor_mul(
                        ctxn[p][qc][64:128, :],
                        ctxn[p][qc][64:128, :], bc[64:128, :],
                    )
                    norm_complete(qc)

                dve_work.append(((p, qc), recq))
                dve_work.append(((p, qc), dmas))
                dve_work.append(((p, qc), muls))

            # ---------------- attention ----------------
            # preload q/k for chunk (0,0) directly; everything else queued
            st = {}
            for quarter in range(4):
                emit_qk_pair(wq_sb, qT, 0, 0, quarter, st)
            st = {}
            for quarter in range(4):
                emit_qk_pair(wk_sb, kT, 0, 0, quarter, st)
            for tb in range(4):
                push_v(tb)
            push_qk(1, 0)

            prev = [None]          # pipelined AV across kb boundaries

            for qc in range(NQC):
                if qc == 1:
                    load_xt(2, nc.sync)
                    load_xt(3, nc.scalar)
                if qc + 1 < NQC:
                    for tb in range(4 * qc + 4, 4 * qc + 8):
                        push_v(tb)
                    push_qk(0, qc + 1)
                    push_qk(1, qc + 1)
                for p in range(2):
                    drain_until([("q", p, qc), ("k", p, qc)])
                    qsl = slice(qc * 512, (qc + 1) * 512)
                    nkb = 4 * qc + 4
                    ctx1 = psp.tile([P, 512], F32, tag="ctx1", bufs=1)
                    ctx2 = psp.tile([P, 512], F32, tag="ctx2", bufs=1)

                    def av(kb, expT, nkb=nkb, p=p, qc=qc,
                           ctx1=ctx1, ctx2=ctx2):
                        st_, sp = kb == 0, kb == nkb - 1
                        oi = kb - 4 * qc
                        off = 128 * oi if oi > 0 else 0
                        nc.tensor.matmul(
                            ctx1[:, off:512], vt[p][kb][:, 0:128],
                            expT[:, off:512],
                            start=st_, stop=sp, skip_group_check=(off > 0),
                        )
                        nc.tensor.matmul(
                            ctx2[:, off:512], vt[p][kb][:, 64:192],
                            expT[:, 512 + off:1024],
                            start=st_, stop=sp, skip_group_check=(off > 0),
                        )
                        if not sp:
                            return
                        # chunk close: den rows + unnormalized ctx -> bf16
                        den = attp.tile([P, 512], F32, tag="den", bufs=2,
                                        name="den")
                        nc.vector.tensor_copy(den[64:65, :], ctx1[64:65, :])
                        nc.vector.tensor_copy(den[32:33, :], ctx2[32:33, :])
                        nc.vector.tensor_copy(
                            ctxn[p][qc][0:64, :], ctx1[0:64, :]
                        )
                        nc.vector.tensor_copy(
                            ctxn[p][qc][64:128, :], ctx2[64:128, :]
                        )
                        push_norm(den, p, qc)

                    for kb in range(nkb):
                        ksl = slice(kb * P, (kb + 1) * P)
                        s_ps = psp.tile([P, 1024], F32, tag="s", bufs=2)
                        nc.tensor.matmul(
                            s_ps[:, 0:512], kT[p][0:64, ksl], qT[p][0:64, qsl],
                            start=True, stop=True, tile_position=(0, 0),
                        )
                        nc.tensor.matmul(
                            s_ps[:, 512:1024], kT[p][64:128, ksl],
                            qT[p][64:128, qsl],
                            start=True, stop=True, tile_position=(64, 0),
                        )
                        oi = kb - 4 * qc
                        off = 128 * oi
                        if oi >= 0:
                            # triangular strip mask on both halves
                            nc.vector.tensor_add(
                                s_ps[:, off:off + 128], s_ps[:, off:off + 128],
                                tri_sb[:],
                            )
                            nc.vector.tensor_add(
                                s_ps[:, 512 + off:640 + off],
                                s_ps[:, 512 + off:640 + off], tri_sb[:],
                            )
                        expT = attp.tile([P, 1024], BF16, tag="exp", bufs=6)
                        nc.scalar.activation(expT[:], s_ps[:], EXP)
                        drain_until([("v", kb)])
                        if prev[0] is not None:
                            prev[0][0](*prev[0][1])
                        prev[0] = (av, (kb, expT))
                        pop_pe(2 if len(pe_work) > 16 else 1)
                        pop_dve()
                        if len(dve_work) > 6:
                            pop_dve()
            if prev[0] is not None:
                drain_until([("v", NTB - 1)])
                prev[0][0](*prev[0][1])
            while dve_work:
                pop_dve()
            while pe_work:
                pop_pe()

    nc.compile()
    return nc


def _get_nc():
    if "nc" not in _CACHE:
        _CACHE["nc"] = _build()
    return _CACHE["nc"]


def kernel(X, W_q, W_k, W_v, W_o, b_o):
    import ml_dtypes
    from concourse.bass_utils import run_bass_kernel_spmd

    BF = ml_dtypes.bfloat16
    X = np.asarray(X, dtype=np.float32)
    # fold the 1/sqrt(head_dim) softmax scale into W_q
    W_q = (np.asarray(W_q, dtype=np.float32) * 0.125).astype(BF)
    W_k = np.asarray(W_k, dtype=np.float32).astype(BF)
    W_v = np.asarray(W_v, dtype=np.float32).astype(BF)
    W_o = np.asarray(W_o, dtype=np.float32).astype(BF)
    b_o = np.asarray(b_o, dtype=np.float32)
    Xb = X.astype(BF)

    nc = _get_nc()
    # triangular strip mask: row kp masks columns j < kp (key > query)
    kp = np.arange(128)[:, None]
    j = np.arange(128)[None, :]
    tri = np.where(kp <= j, 0.0, NEG).astype(np.float32)

    in_maps = []
    for c in range(NCORES):
        b = c // 4
        hb = c % 4
        cs = slice(hb * CLOC, (hb + 1) * CLOC)
        xt = Xb[b].T  # [1024, 2048]
        xr = np.ascontiguousarray(
            xt.reshape(NCB, 128, NQC, 512).transpose(2, 0, 1, 3)
        ).reshape(NQC * NCB * 128, 512)

        def wrearr(W):
            # [1024, 256] -> [128, 8*256] cb-major columns
            return np.ascontiguousarray(
                W.reshape(NCB, 128, CLOC).transpose(1, 0, 2)
            ).reshape(128, NCB * CLOC)

        in_maps.append({
            "Xr": xr,
            "Wq": wrearr(W_q[:, cs]),
            "Wk": wrearr(W_k[:, cs]),
            "Wv": wrearr(W_v[:, cs]),
            "Wo": np.ascontiguousarray(W_o[cs, :]),
            "tri": tri,
        })

    global _last_in_maps
    _last_in_maps = in_maps
    res = run_bass_kernel_spmd(nc, in_maps, list(range(NCORES)))
    out = np.empty((B, NTOK, DIN), dtype=np.float32)
    for b in range(B):
        acc = res.results[4 * b]["out"].astype(np.float32)
        for hb in range(1, 4):
            acc = acc + res.results[4 * b + hb]["out"].astype(np.float32)
        out[b] = acc + b_o[None, :]
    return out


# revision 29
# speedup vs baseline: 1.0458x; 1.0458x over previous
"""Multi-head causal attention on 8 Trainium2 NeuronCores.

Problem: X [2, 2048, 1024] f32, W_q/W_k/W_v [1024, 1024], W_o [1024, 1024],
b_o [1024]; 16 heads, head_dim 64, causal softmax attention + out projection.

Sharding: 2 (batch) x 4 (head-blocks of 4 heads) = 8 cores. Each core
computes q/k/v for its 4 heads on its batch, causal attention, and a partial
output projection ctx @ W_o[rows]. Host sums the 4 partials per batch and
adds b_o. No cross-core collectives.

Single-pass structure (one pool scope; the Tile scheduler interleaves, with
emission order as priority):
  - X arrives host-transposed in (qc, cb)-block-contiguous bf16 so XT tiles
    are single contiguous DMA loads; weights are one wide DMA each, all
    ordered by first need (front-critical on sync, bulk deferred / scalar).
    W_q carries the 1/sqrt(d) softmax scale.
  - QKV projection and output-projection work is queued as fine-grained
    (<=2 matmul) closures popped one per attention k-block iteration, so
    the in-order PE queue interleaves them into the scores->exp->AV stream
    (exp on ScalarE, (N+352)/1.2 ns, is the attention-stream floor);
    drain_until gates guarantee producers are emitted before consumers.
  - Scores: row-packed concurrent K=64 matmul pairs (tile_position 0/64)
    into [128,1024] PSUM; strip mask added on DVE for diagonal blocks; one
    full-width exp per k-block.
  - AV: vt [vA|ones|pad|ones|pad|vB] sliding-window trick accumulates both
    heads' ctx and the softmax denominators in two PSUM banks, skipping
    fully-masked columns.
  - Normalization deferred off the critical path (one DVE-op per iteration):
    den rows -> chunked DVE reciprocal (ACT ln/exp(-x) for the final chunk,
    where ACT is idle) -> DRAM round-trip broadcast -> in-place bf16 muls
    on the unnormalized ctxn; outproj tiles become PE filler when a
    q-chunk's two pairs are normalized.
  - PSUM: s x2 (4 banks) + ctx1 + ctx2 + fill x2 (qkv/outproj shared,
    double-buffered) = 8 banks. ScalarE kept exp-only during the stream;
    copies pinned to DVE; vt ones/pads via gpsimd memset; output stores
    alternate between the sync and scalar DMA queues.
"""
import sys

sys.path.insert(0, "/opt/trn_rl_repo")

import numpy as np

NEG = -1.0e9
B, NTOK, DIN = 2, 2048, 1024
NH, HD = 16, 64
HPC = 4            # heads per core
CLOC = HPC * HD    # 256 local channels
NCORES = 8
NTB = NTOK // 128  # 16 token blocks
NQC = NTOK // 512  # 4 q-chunks
NCB = DIN // 128   # 8 contraction blocks

_CACHE = {}
_last_in_maps = None


def _build():
    from concourse import bacc, mybir, tile

    F32 = mybir.dt.float32
    BF16 = mybir.dt.bfloat16
    EXP = mybir.ActivationFunctionType.Exp
    P = 128

    nc = bacc.Bacc(None)
    # host-transposed X in (qc, cb)-block-contiguous layout: block (qc, cb)
    # at rows [(qc*NCB+cb)*128, +128) is XT[cb*128:(cb+1)*128, qc*512:+512]
    Xr = nc.declare_dram_parameter("Xr", [NQC * NCB * P, 512], BF16,
                                   isOutput=False)
    # weights cb-major along columns: [:, cb*256:(cb+1)*256] = W[cb block]
    Wq = nc.declare_dram_parameter("Wq", [P, NCB * CLOC], BF16, isOutput=False)
    Wk = nc.declare_dram_parameter("Wk", [P, NCB * CLOC], BF16, isOutput=False)
    Wv = nc.declare_dram_parameter("Wv", [P, NCB * CLOC], BF16, isOutput=False)
    Wo = nc.declare_dram_parameter("Wo", [CLOC, DIN], BF16, isOutput=False)
    tri = nc.declare_dram_parameter("tri", [P, P], BF16, isOutput=False)
    out = nc.declare_dram_parameter("out", [NTOK, DIN], BF16, isOutput=True)

    with tile.TileContext(nc) as tc:
        with (
            tc.tile_pool(name="const", bufs=1) as constp,
            tc.tile_pool(name="xt", bufs=1) as xtp,
            tc.tile_pool(name="w", bufs=1) as wp,
            tc.tile_pool(name="qkT", bufs=1) as qkTp,
            tc.tile_pool(name="vt", bufs=1) as vtp,
            tc.tile_pool(name="ctxn", bufs=1) as ctxnp,
            tc.tile_pool(name="att", bufs=1) as attp,
            tc.tile_pool(name="osb", bufs=1) as osbp,
            tc.tile_pool(name="dsc", bufs=1, space="DRAM") as dscp,
            tc.tile_pool(name="ps", bufs=1, space="PSUM") as psp,
        ):
            tri_sb = constp.tile([P, P], BF16, tag="tri")
            nc.sync.dma_start(tri_sb[:], tri[:])

            # weights: one wide tile per matrix, single contiguous DMA each
            wq_sb = wp.tile([P, NCB * CLOC], BF16, tag="wq", name="wq")
            wk_sb = wp.tile([P, NCB * CLOC], BF16, tag="wk", name="wk")
            wv_sb = wp.tile([P, NCB * CLOC], BF16, tag="wv", name="wv")
            wo_sb = [wp.tile([P, DIN], BF16, tag=f"wo{p}", name=f"wo{p}")
                     for p in range(2)]
            XT = [
                [xtp.tile([P, 512], BF16, tag=f"xt{cb}_{q}", name=f"xt{cb}_{q}")
                 for q in range(NQC)]
                for cb in range(NCB)
            ]

            def load_xt(q, eng):
                for cb in range(NCB):
                    r0 = (q * NCB + cb) * P
                    eng.dma_start(XT[cb][q][:], Xr[r0:r0 + P, :])

            # ordered by first need; front-critical loads on sync, bulk on
            # the scalar queue (idle until the first exp ~16us in)
            nc.sync.dma_start(wq_sb[:], Wq[:])
            nc.scalar.dma_start(wv_sb[:], Wv[:])
            load_xt(0, nc.sync)
            nc.sync.dma_start(wk_sb[:], Wk[:])
            for p in range(2):
                nc.scalar.dma_start(wo_sb[p][:], Wo[p * P:(p + 1) * P, :])
            load_xt(1, nc.sync)

            # static result tiles
            qT = [qkTp.tile([P, NTOK], BF16, tag=f"qT{p}", name=f"qT{p}")
                  for p in range(2)]
            kT = [qkTp.tile([P, NTOK], BF16, tag=f"kT{p}", name=f"kT{p}")
                  for p in range(2)]
            # v tiles [128 keys, 192]: [vA | ones | pad | ones | pad | vB]
            vt = [
                [vtp.tile([P, 192], BF16, tag=f"vt{p}_{tb}", name=f"vt{p}_{tb}")
                 for tb in range(NTB)]
                for p in range(2)
            ]
            ctxn = [
                [ctxnp.tile([P, 512], BF16, tag=f"ctxn{p}_{qc}",
                            name=f"ctxn{p}_{qc}") for qc in range(NQC)]
                for p in range(2)
            ]
            for p in range(2):
                for tb in range(NTB):
                    t = vt[p][tb]
                    nc.gpsimd.memset(t[:, 64:128], 0.0)
                    nc.gpsimd.memset(t[:, 64:65], 1.0)
                    nc.gpsimd.memset(t[:, 96:97], 1.0)

            # ---------------- emission helpers ----------------
            # pe_work / dve_work: queues of (key, closure); each closure is
            # <=2 matmuls (or one DVE op chain link) so the in-order engine
            # queues interleave finely with the scores/exp/AV stream.
            pe_work = []
            dve_work = []
            emitted = {}   # key -> remaining closures not yet emitted

            def push_pe(key, fn):
                pe_work.append((key, fn))
                emitted[key] = emitted.get(key, 0) + 1

            def pop_pe(n=1):
                for _ in range(n):
                    if not pe_work:
                        return
                    k, fn = pe_work.pop(0)
                    fn()
                    emitted[k] -= 1

            def pop_dve():
                if dve_work:
                    dve_work.pop(0)[1]()

            def drain_until(keys):
                need = [k for k in keys if emitted.get(k, 0) > 0]
                while need:
                    k, fn = pe_work.pop(0)
                    fn()
                    emitted[k] -= 1
                    need = [k for k in keys if emitted.get(k, 0) > 0]

            def emit_qk_pair(w_sb, dst, p, qc, quarter, state):
                if quarter == 0:
                    state["t"] = psp.tile([P, 512], F32, tag="fill", bufs=2,
                                          name="qk_ps")
                t = state["t"]
                for cb in range(2 * quarter, 2 * quarter + 2):
                    csl = slice(cb * CLOC + p * P, cb * CLOC + (p + 1) * P)
                    nc.tensor.matmul(
                        t[:], w_sb[:, csl], XT[cb][qc][:],
                        start=(cb == 0), stop=(cb == NCB - 1),
                    )
                if quarter == 3:
                    nc.vector.tensor_copy(
                        dst[p][:, qc * 512:(qc + 1) * 512], t[:]
                    )

            def emit_v_pair(tb, quarter, state):
                if quarter == 0:
                    state["t"] = psp.tile([P, 512], F32, tag="fill", bufs=2,
                                          name="v_ps")
                t = state["t"]
                for cb in range(2 * quarter, 2 * quarter + 2):
                    nc.tensor.matmul(
                        t[:, 0:CLOC],
                        XT[cb][tb // 4][:, (tb % 4) * P:(tb % 4 + 1) * P],
                        wv_sb[:, cb * CLOC:(cb + 1) * CLOC],
                        start=(cb == 0), stop=(cb == NCB - 1),
                    )
                if quarter == 3:
                    for p in range(2):
                        hA, hB = 2 * p, 2 * p + 1
                        nc.vector.tensor_copy(
                            vt[p][tb][:, 0:64], t[:, hA * 64:(hA + 1) * 64]
                        )
                        nc.vector.tensor_copy(
                            vt[p][tb][:, 128:192], t[:, hB * 64:(hB + 1) * 64]
                        )

            def push_qk(p, qc):
                for nm, w_sb, dst in (("q", wq_sb, qT), ("k", wk_sb, kT)):
                    st = {}
                    for quarter in range(4):
                        push_pe(
                            (nm, p, qc),
                            lambda w_sb=w_sb, dst=dst, p=p, qc=qc,
                            quarter=quarter, st=st:
                            emit_qk_pair(w_sb, dst, p, qc, quarter, st)
                        )

            def push_v(tb):
                st = {}
                for quarter in range(4):
                    push_pe(
                        ("v", tb),
                        lambda tb=tb, quarter=quarter, st=st:
                        emit_v_pair(tb, quarter, st)
                    )

            def emit_outproj_tile(qc, i, jc):
                tb = 4 * qc + i
                tsl = slice(tb * P, (tb + 1) * P)
                jsl = slice(jc * 512, (jc + 1) * 512)
                o_ps = psp.tile([P, 512], F32, tag="fill", bufs=2,
                                name="o_ps")
                for pp in range(2):
                    nc.tensor.matmul(
                        o_ps[:],
                        ctxn[pp][qc][:, i * P:(i + 1) * P],
                        wo_sb[pp][:, jsl],
                        start=(pp == 0), stop=(pp == 1),
                    )
                o_sb = osbp.tile([P, 512], BF16, tag="o_sb", bufs=4,
                                 name="o_sb")
                nc.vector.tensor_copy(o_sb[:], o_ps[:])
                (nc.sync if (i + jc) % 2 == 0 else nc.scalar).dma_start(
                    out[tsl, jsl], o_sb[:]
                )

            # norm bookkeeping: when both chunks of a qc have their norm
            # fully emitted, its outproj tiles become pe filler
            norm_done = [0, 0, 0, 0]

            def norm_complete(qc):
                norm_done[qc] += 1
                if norm_done[qc] == 2:
                    for i in range(4):
                        for jc in range(2):
                            push_pe(
                                ("o", qc, i, jc),
                                lambda qc=qc, i=i, jc=jc:
                                emit_outproj_tile(qc, i, jc)
                            )

            LOG = mybir.ActivationFunctionType.Ln

            def push_norm(den, p, qc):
                rec = attp.tile([P, 512], F32, tag="rec", bufs=2, name="rec")
                bc = attp.tile([P, 512], BF16, tag="bc", bufs=2, name="bc")
                d_t = dscp.tile([2, 512], BF16, tag="d", bufs=2, name="d_t")

                last = (p, qc) == (1, NQC - 1)
                recb = attp.tile([P, 512], BF16, tag="recb", bufs=2,
                                 name="recb")

                def recq(rec=rec, recb=recb, den=den, last=last):
                    if last:
                        # tail: ACT is idle; 1/d = exp(-ln d), 32-aligned
                        # windows (rows 33..63 / 65..95 are garbage, unread)
                        nc.scalar.activation(rec[32:64, :], den[32:64, :], LOG)
                        nc.scalar.activation(recb[32:64, :], rec[32:64, :],
                                             EXP, scale=-1.0)
                        nc.scalar.activation(rec[64:96, :], den[64:96, :], LOG)
                        nc.scalar.activation(recb[64:96, :], rec[64:96, :],
                                             EXP, scale=-1.0)
                    else:
                        for rc in range(4):
                            rsl = slice(rc * 128, (rc + 1) * 128)
                            nc.vector.reciprocal(rec[:, rsl], den[:, rsl])
                        nc.vector.tensor_copy(recb[64:65, :], rec[64:65, :])
                        nc.vector.tensor_copy(recb[32:33, :], rec[32:33, :])

                def dmas(recb=recb, d_t=d_t, bc=bc):
                    nc.sync.dma_start(d_t[0:1, :], recb[64:65, :])
                    nc.sync.dma_start(d_t[1:2, :], recb[32:33, :])
                    nc.sync.dma_start(
                        bc[0:64, :], d_t[0:1, :].to_broadcast((64, 512))
                    )
                    nc.sync.dma_start(
                        bc[64:128, :], d_t[1:2, :].to_broadcast((64, 512))
                    )

                def muls(bc=bc, p=p, qc=qc):
                    nc.vector.tensor_mul(
                        ctxn[p][qc][0:64, :], ctxn[p][qc][0:64, :],
                        bc[0:64, :],
                    )
                    nc.vector.tensor_mul(
                        ctxn[p][qc][64:128, :],
                        ctxn[p][qc][64:128, :], bc[64:128, :],
                    )
                    norm_complete(qc)

                dve_work.append(((p, qc), recq))
                dve_work.append(((p, qc), dmas))
                dve_work.append(((p, qc), muls))

            # ---------------- attention ----------------
            # preload q/k for chunk (0,0) directly; everything else queued
            st = {}
            for quarter in range(4):
                emit_qk_pair(wq_sb, qT, 0, 0, quarter, st)
            st = {}
            for quarter in range(4):
                emit_qk_pair(wk_sb, kT, 0, 0, quarter, st)
            for tb in range(4):
                push_v(tb)
            push_qk(1, 0)

            prev = [None]          # pipelined AV across kb boundaries

            for qc in range(NQC):
                if qc == 1:
                    load_xt(2, nc.sync)
                    load_xt(3, nc.scalar)
                if qc + 1 < NQC:
                    for tb in range(4 * qc + 4, 4 * qc + 8):
                        push_v(tb)
                    push_qk(0, qc + 1)
                    push_qk(1, qc + 1)
                for p in range(2):
                    drain_until([("q", p, qc), ("k", p, qc)])
                    qsl = slice(qc * 512, (qc + 1) * 512)
                    nkb = 4 * qc + 4
                    ctx1 = psp.tile([P, 512], F32, tag="ctx1", bufs=1)
                    ctx2 = psp.tile([P, 512], F32, tag="ctx2", bufs=1)

                    def av(kb, expT, nkb=nkb, p=p, qc=qc,
                           ctx1=ctx1, ctx2=ctx2):
                        st_, sp = kb == 0, kb == nkb - 1
                        oi = kb - 4 * qc
                        off = 128 * oi if oi > 0 else 0
                        nc.tensor.matmul(
                            ctx1[:, off:512], vt[p][kb][:, 0:128],
                            expT[:, off:512],
                            start=st_, stop=sp, skip_group_check=(off > 0),
                        )
                        nc.tensor.matmul(
                            ctx2[:, off:512], vt[p][kb][:, 64:192],
                            expT[:, 512 + off:1024],
                            start=st_, stop=sp, skip_group_check=(off > 0),
                        )
                        if not sp:
                            return
                        # chunk close: den rows + unnormalized ctx -> bf16
                        den = attp.tile([P, 512], F32, tag="den", bufs=2,
                                        name="den")
                        nc.vector.tensor_copy(den[64:65, :], ctx1[64:65, :])
                        nc.vector.tensor_copy(den[32:33, :], ctx2[32:33, :])
                        nc.vector.tensor_copy(
                            ctxn[p][qc][0:64, :], ctx1[0:64, :]
                        )
                        nc.vector.tensor_copy(
                            ctxn[p][qc][64:128, :], ctx2[64:128, :]
                        )
                        push_norm(den, p, qc)

                    for kb in range(nkb):
                        ksl = slice(kb * P, (kb + 1) * P)
                        s_ps = psp.tile([P, 1024], F32, tag="s", bufs=2)
                        nc.tensor.matmul(
                            s_ps[:, 0:512], kT[p][0:64, ksl], qT[p][0:64, qsl],
                            start=True, stop=True, tile_position=(0, 0),
                        )
                        nc.tensor.matmul(
                            s_ps[:, 512:1024], kT[p][64:128, ksl],
                            qT[p][64:128, qsl],
                            start=True, stop=True, tile_position=(64, 0),
                        )
                        oi = kb - 4 * qc
                        off = 128 * oi
                        expT = attp.tile([P, 1024], BF16, tag="exp", bufs=6)
                        nc.scalar.activation(expT[:], s_ps[:], EXP)
                        if oi >= 0:
                            # causal strip: zero masked weights post-exp on
                            # the (idle) gpsimd engine, off the DVE chain
                            nc.gpsimd.tensor_mul(
                                expT[:, off:off + 128],
                                expT[:, off:off + 128], tri_sb[:],
                            )
                            nc.gpsimd.tensor_mul(
                                expT[:, 512 + off:640 + off],
                                expT[:, 512 + off:640 + off], tri_sb[:],
                            )
                        drain_until([("v", kb)])
                        if prev[0] is not None:
                            prev[0][0](*prev[0][1])
                        prev[0] = (av, (kb, expT))
                        pop_pe(2 if len(pe_work) > 16 else 1)
                        pop_dve()
                        if len(dve_work) > 6:
                            pop_dve()
            if prev[0] is not None:
                drain_until([("v", NTB - 1)])
                prev[0][0](*prev[0][1])
            while dve_work:
                pop_dve()
            while pe_work:
                pop_pe()

    nc.compile()
    return nc


def _get_nc():
    if "nc" not in _CACHE:
        _CACHE["nc"] = _build()
    return _CACHE["nc"]


def kernel(X, W_q, W_k, W_v, W_o, b_o):
    import ml_dtypes
    from concourse.bass_utils import run_bass_kernel_spmd

    BF = ml_dtypes.bfloat16
    X = np.asarray(X, dtype=np.float32)
    # fold the 1/sqrt(head_dim) softmax scale into W_q
    W_q = (np.asarray(W_q, dtype=np.float32) * 0.125).astype(BF)
    W_k = np.asarray(W_k, dtype=np.float32).astype(BF)
    W_v = np.asarray(W_v, dtype=np.float32).astype(BF)
    W_o = np.asarray(W_o, dtype=np.float32).astype(BF)
    b_o = np.asarray(b_o, dtype=np.float32)
    Xb = X.astype(BF)

    nc = _get_nc()
    # triangular strip mask: row kp keeps columns j >= kp (0/1, post-exp)
    kp = np.arange(128)[:, None]
    j = np.arange(128)[None, :]
    tri = np.where(kp <= j, 1.0, 0.0).astype(ml_dtypes.bfloat16)

    in_maps = []
    for c in range(NCORES):
        b = c // 4
        hb = c % 4
        cs = slice(hb * CLOC, (hb + 1) * CLOC)
        xt = Xb[b].T  # [1024, 2048]
        xr = np.ascontiguousarray(
            xt.reshape(NCB, 128, NQC, 512).transpose(2, 0, 1, 3)
        ).reshape(NQC * NCB * 128, 512)

        def wrearr(W):
            # [1024, 256] -> [128, 8*256] cb-major columns
            return np.ascontiguousarray(
                W.reshape(NCB, 128, CLOC).transpose(1, 0, 2)
            ).reshape(128, NCB * CLOC)

        in_maps.append({
            "Xr": xr,
            "Wq": wrearr(W_q[:, cs]),
            "Wk": wrearr(W_k[:, cs]),
            "Wv": wrearr(W_v[:, cs]),
            "Wo": np.ascontiguousarray(W_o[cs, :]),
            "tri": tri,
        })

    global _last_in_maps
    _last_in_maps = in_maps
    res = run_bass_kernel_spmd(nc, in_maps, list(range(NCORES)))
    out = np.empty((B, NTOK, DIN), dtype=np.float32)
    for b in range(B):
        acc = res.results[4 * b]["out"].astype(np.float32)
        for hb in range(1, 4):
            acc = acc + res.results[4 * b + hb]["out"].astype(np.float32)
        out[b] = acc + b_o[None, :]
    return out
